# revision 22
# baseline (speedup 1.0000x reference)
"""Trainium2 Bass kernel for nn_MultiHeadAttn (B=4, S=2048, D=1024, H=16).

Sharding: 8 cores = 4 batches x 2 head-groups (tensor-parallel over heads).
Each core computes one batch's attention for 8 of 16 heads (512 of 1024
feature dims) and a partial output projection; the host sums the two
head-group partials per batch (the "all-reduce" of row-parallel Wo).

Device dataflow (matmuls in fp16 with fp32 PSUM accumulation; fp16 keeps
10 mantissa bits so end-to-end error stays ~7e-4 while enabling the fast
weight-load path the PE lacks for fp32/fp32r):
  - Host pre-transposes activations (q/k/v -> [D, S]) and weight slices and
    converts to fp16, so the kernel needs no on-device transposes or
    casting DMAs.
  - QT/KT computed feature-major [512, 2048]; V computed token-major with a
    ones column per head ([128, 8*65] tiles) so the attn@V matmul (M=65)
    also produces the softmax row-sums.
  - Scores computed transposed S^T[k,q] with 2-head row-tiled matmuls
    (K=64 pairs packed at tile_position (0,0)/(64,0)).
  - softmax without max-subtraction (scores/8 ~ N(0,1), exp is safe);
    exp on ScalarE with scale=1/8 fused.
  - Each attention tile emits its 16 exp+next-scores pairs first (the
    ScalarE chain), then filler projection work (V / later-tile Q), then
    the 32 attn@V matmuls consuming a deep P pool — so ScalarE, the
    critical engine, is never queued behind PE filler work.
  - Division: K=1 ones-matmul broadcast of raw row-sums + fast DVE
    reciprocal (reciprocal_approx_fast, ~18-bit) + DVE multiply, inline at
    tile end reading Y straight from PSUM.
  - Output projection consumes X^T directly; bv/bo folded into a single
    host-precomputed effective bias; emitted per token-tile right after
    the last head-pair finishes that tile.
"""
import numpy as np

B, S, D = 4, 2048, 1024
H = 16
DK = 64
G = 2              # head groups (tensor-parallel factor)
DL = D // G        # 512 local feature dims per core
NHL = H // G       # 8 local heads
NJ = NHL // 2      # 4 head pairs
NT = S // 512      # 4 token tiles of 512
NKC = S // 128     # 16 k-token chunks of 128
NDC = D // 128     # 8 d_in chunks
NM = DL // 128     # 4 local out chunks
NMO = D // 128     # 8 output d chunks

_CACHED = {}


def _build_nc():
    import concourse.bass as bass
    import concourse.tile as tile
    from concourse import bacc, mybir

    FP32 = mybir.dt.float32
    FP16 = mybir.dt.float16
    AF = mybir.ActivationFunctionType
    ts = bass.ts

    nc = bacc.Bacc(None, target_bir_lowering=False, debug=False)

    qT_d = nc.dram_tensor("qT", [D, S], FP16, kind="ExternalInput")
    kT_d = nc.dram_tensor("kT", [D, S], FP16, kind="ExternalInput")
    vT_d = nc.dram_tensor("vT", [D, S], FP16, kind="ExternalInput")
    wqT_d = nc.dram_tensor("wqT", [D, DL], FP16, kind="ExternalInput")
    wkT_d = nc.dram_tensor("wkT", [D, DL], FP16, kind="ExternalInput")
    wvT_d = nc.dram_tensor("wvT", [D, DL], FP16, kind="ExternalInput")
    woT_d = nc.dram_tensor("woT", [DL, D], FP16, kind="ExternalInput")
    bq_d = nc.dram_tensor("bq", [NM, 128, 1], FP32, kind="ExternalInput")
    bk_d = nc.dram_tensor("bk", [NM, 128, 1], FP32, kind="ExternalInput")
    bo_d = nc.dram_tensor("bo", [NMO, 128, 1], FP32, kind="ExternalInput")
    out_d = nc.dram_tensor("outT", [D, S], FP32, kind="ExternalOutput")

    with tile.TileContext(nc) as tc:
        with (
            tc.tile_pool(name="const", bufs=1) as const,
            tc.tile_pool(name="wflat", bufs=24) as wflat,
            tc.tile_pool(name="wop", bufs=4) as wop,
            tc.tile_pool(name="qkwin", bufs=16) as qkwin,
            tc.tile_pool(name="vtwin", bufs=12) as vtwin,
            tc.tile_pool(name="big", bufs=1) as big,
            tc.tile_pool(name="vaug", bufs=1) as vaug,
            tc.tile_pool(name="ppool", bufs=18) as ppool,
            tc.tile_pool(name="small", bufs=3) as small,
            tc.tile_pool(name="outst", bufs=3) as outst,
            tc.tile_pool(name="ps_mm", bufs=2, space="PSUM") as ps_mm,
            tc.tile_pool(name="ps_s", bufs=2, space="PSUM") as ps_s,
            tc.tile_pool(name="ps_y", bufs=2, space="PSUM") as ps_y,
        ):
            # ---- constants
            onescols = const.tile([128, NHL, 1], FP16, name="onescols")
            nc.vector.memset(onescols[:], 1.0)
            bq_sb, bk_sb, bo_sb = [], [], []
            for m in range(NM):
                t_ = const.tile([128, 1], FP32, name=f"bq{m}")
                nc.gpsimd.dma_start(t_[:], bq_d[m])
                bq_sb.append(t_)
                t_ = const.tile([128, 1], FP32, name=f"bk{m}")
                nc.gpsimd.dma_start(t_[:], bk_d[m])
                bk_sb.append(t_)
            for m in range(NMO):
                t_ = const.tile([128, 1], FP32, name=f"bo{m}")
                nc.gpsimd.dma_start(t_[:], bo_d[m])
                bo_sb.append(t_)

            # ---- weight tiles (loads emitted in the emission section so
            # the first projection's windows aren't queued behind them)
            wq_sb, wk_sb, wv_sb, wo_sb = [], [], [], []
            for kc in range(NDC):
                wk_sb.append(wflat.tile([128, DL], FP16, tag="w",
                                        name=f"wk{kc}"))
                wq_sb.append(wflat.tile([128, DL], FP16, tag="w",
                                        name=f"wq{kc}"))
                wv_sb.append(wflat.tile([128, DL], FP16, tag="w",
                                        name=f"wv{kc}"))
            for jc in range(NJ):
                wo_sb.append(wop.tile([128, D], FP16, tag="wo",
                                      name=f"wo{jc}"))

            # ---- resident activation tiles (fp16)
            QT = [big.tile([128, S], FP16, name=f"QT{m}") for m in range(NM)]
            KT = [big.tile([128, S], FP16, name=f"KT{m}") for m in range(NM)]
            X = [big.tile([128, S], FP16, name=f"X{j}") for j in range(NJ)]
            VA = [vaug.tile([128, NHL * 65], FP16, name=f"va{c}")
                  for c in range(NKC)]
            va_view = [va[:].rearrange("p (h c) -> p h c", c=65) for va in VA]

            # ---- task emitters -------------------------------------------
            def qk_task(src_d, w_sb, b_sb, dst, t, ms):
                """Project token-tile t of q or k for m-chunks in ms."""
                win = []
                for kc in range(NDC):
                    w_ = qkwin.tile([128, 512], FP16, tag="win",
                                    name=f"win{kc}")
                    nc.gpsimd.dma_start(w_[:], src_d[ts(kc, 128), ts(t, 512)])
                    win.append(w_)
                for m in ms:
                    ps = ps_mm.tile([128, 512], FP32, tag="mm", name="psA")
                    for kc in range(NDC):
                        nc.tensor.matmul(
                            ps[:], w_sb[kc][:, ts(m, 128)], win[kc][:],
                            start=(kc == 0), stop=(kc == NDC - 1))
                    nc.vector.tensor_scalar_add(
                        dst[m][:, ts(t, 512)], ps[:], b_sb[m][:])

            def v_task(c):
                """Project token-chunk c of v into the ones-augmented VA."""
                ps = ps_mm.tile([128, 512], FP32, tag="mm", name="psV")
                for kc in range(NDC):
                    vt = vtwin.tile([128, 128], FP16, tag="vt", name="vt")
                    nc.gpsimd.dma_start(vt[:], vT_d[ts(kc, 128), ts(c, 128)])
                    nc.tensor.matmul(ps[:], vt[:], wv_sb[kc][:],
                                     start=(kc == 0), stop=(kc == NDC - 1))
                ps_v = ps[:].rearrange("p (h c) -> p h c", c=64)
                nc.vector.tensor_copy(va_view[c][:, :, 0:64], ps_v)
                nc.vector.tensor_copy(va_view[c][:, :, 64:65], onescols[:])

            def out_task(t):
                """Output projection for token-tile t (needs all X_j)."""
                for m in range(NMO):
                    ps = ps_mm.tile([128, 512], FP32, tag="mm", name="psO")
                    for j in range(NJ):
                        nc.tensor.matmul(
                            ps[:], wo_sb[j][:, ts(m, 128)],
                            X[j][:, ts(t, 512)],
                            start=(j == 0), stop=(j == NJ - 1))
                    st = outst.tile([128, 512], FP32, tag="st", name="st")
                    nc.vector.tensor_scalar_add(st[:], ps[:], bo_sb[m][:])
                    nc.sync.dma_start(out_d[ts(m, 128), ts(t, 512)], st[:])

            # ---- deferred K/Q projection fillers, (m, t)-granular.
            # Pair j needs KT_j / QT_j (m-chunk j) for all t; m=0 runs
            # upfront, m-chunk j is drained as filler during pair j-1.
            done = set()

            def ensure_proj(which, m, t):
                if (which, m, t) in done:
                    return
                done.add((which, m, t))
                if which == "k":
                    qk_task(kT_d, wk_sb, bk_sb, KT, t, [m])
                else:
                    qk_task(qT_d, wq_sb, bq_sb, QT, t, [m])

            filler_q = [(w, m, t) for m in range(1, NM)
                        for t in range(NT) for w in ("k", "q")]

            def pop_filler():
                while filler_q:
                    w, m, t = filler_q.pop(0)
                    if (w, m, t) not in done:
                        ensure_proj(w, m, t)
                        return

            def attn_tile(j, t):
                """Attention for head-pair j, token-tile t.
                Emission order: the full exp/scores chain, then fillers,
                then the attn@V matmuls, then the normalization."""
                first = (j == 0 and t == 0)
                ys = [ps_y.tile([65, 512], FP32, tag="y", name=f"y{h}")
                      for h in range(2)]

                def scores(k):
                    s_ps = ps_s.tile([128, 1024], FP32, tag="s", name="s")
                    nc.tensor.matmul(
                        s_ps[:, 0:512], KT[j][0:64, ts(k, 128)],
                        QT[j][0:64, ts(t, 512)],
                        start=True, stop=True, tile_position=(0, 0))
                    nc.tensor.matmul(
                        s_ps[:, 512:1024], KT[j][64:128, ts(k, 128)],
                        QT[j][64:128, ts(t, 512)],
                        start=True, stop=True, tile_position=(64, 0))
                    return s_ps

                def a_v(k, p):
                    for h in range(2):
                        nc.tensor.matmul(
                            ys[h][:],
                            VA[k][:, 65 * (2 * j + h):65 * (2 * j + h) + 65],
                            p[:, 512 * h:512 * (h + 1)],
                            start=(k == 0), stop=(k == NKC - 1))

                # attn@V runs two iterations behind exp so it never waits on
                # the exp semaphore round-trip
                if first:
                    v_task(0)
                s_cur = scores(0)
                plag = []
                for k in range(NKC):
                    p = ppool.tile([128, 1024], FP16, tag="p", name="p")
                    nc.scalar.activation(p[:], s_cur[:], AF.Exp, scale=0.125)
                    plag.append((k, p))
                    if k + 1 < NKC:
                        s_cur = scores(k + 1)
                    if first and k + 1 < NKC:
                        v_task(k + 1)
                    elif k in (3, 8, 13):
                        pop_filler()
                    if len(plag) > 2:
                        a_v(*plag.pop(0))
                while plag:
                    a_v(*plag.pop(0))

                # normalization straight from PSUM; the row-sum broadcast
                # runs on the otherwise-idle GpSimd engine so the PE rolls
                # directly into the next tile
                for h in range(2):
                    rs = small.tile([1, 512], FP32, tag="rs", name="rs")
                    nc.vector.tensor_copy(rs[:], ys[h][64:65, :])
                    rbb = small.tile([64, 512], FP32, tag="rbb", name="rbb")
                    nc.gpsimd.partition_broadcast(rbb[:], rs[:], channels=64)
                    ri = small.tile([64, 512], FP32, tag="ri", name="ri")
                    nc.vector.reciprocal_approx_fast(ri[:], rbb[:])
                    nc.vector.tensor_mul(
                        X[j][64 * h:64 * h + 64, ts(t, 512)],
                        ys[h][0:64, :], ri[:])

            # ---- emission ------------------------------------------------
            for kc in range(NDC):
                nc.sync.dma_start(wk_sb[kc][:], wkT_d[ts(kc, 128), :])
            for t in range(NT):
                ensure_proj("k", 0, t)
            for kc in range(NDC):
                nc.sync.dma_start(wq_sb[kc][:], wqT_d[ts(kc, 128), :])
            for t in range(NT):
                ensure_proj("q", 0, t)
            for kc in range(NDC):
                nc.sync.dma_start(wv_sb[kc][:], wvT_d[ts(kc, 128), :])
            for jc in range(NJ):
                nc.sync.dma_start(wo_sb[jc][:], woT_d[ts(jc, 128), :])

            for j in range(NJ):
                for t in range(NT):
                    ensure_proj("k", j, t)
                    ensure_proj("q", j, t)
                    attn_tile(j, t)
                    if j == NJ - 1:
                        out_task(t)

    nc.compile()
    return nc


def _prep_in_maps(q, k, v, Wq, bq, Wk, bk, Wv, bv, Wo, bo):
    f16 = np.float16
    in_maps = []
    for core in range(8):
        b, g = divmod(core, G)
        rows = slice(DL * g, DL * (g + 1))
        bo_eff = Wo[:, rows].astype(np.float32) @ bv[rows].astype(np.float32)
        if g == 0:
            bo_eff = bo_eff + bo
        in_maps.append({
            "qT": np.ascontiguousarray(q[b].T.astype(f16)),
            "kT": np.ascontiguousarray(k[b].T.astype(f16)),
            "vT": np.ascontiguousarray(v[b].T.astype(f16)),
            "wqT": np.ascontiguousarray(Wq[rows, :].T.astype(f16)),
            "wkT": np.ascontiguousarray(Wk[rows, :].T.astype(f16)),
            "wvT": np.ascontiguousarray(Wv[rows, :].T.astype(f16)),
            "woT": np.ascontiguousarray(Wo[:, rows].T.astype(f16)),
            "bq": np.ascontiguousarray(bq[rows].reshape(NM, 128, 1)),
            "bk": np.ascontiguousarray(bk[rows].reshape(NM, 128, 1)),
            "bo": np.ascontiguousarray(
                bo_eff.astype(np.float32).reshape(NMO, 128, 1)),
        })
    return in_maps


def kernel(q, k, v, mask, Wq, bq, Wk, bk, Wv, bv, Wo, bo,
           _trace=False, _tmpdir=None):
    from concourse.bass_utils import run_bass_kernel_spmd

    q, k, v = (np.asarray(x, dtype=np.float32) for x in (q, k, v))
    Wq, bq, Wk, bk, Wv, bv, Wo, bo = (
        np.asarray(x, dtype=np.float32)
        for x in (Wq, bq, Wk, bk, Wv, bv, Wo, bo))

    if "nc" not in _CACHED:
        _CACHED["nc"] = _build_nc()
    nc = _CACHED["nc"]

    in_maps = _prep_in_maps(q, k, v, Wq, bq, Wk, bk, Wv, bv, Wo, bo)
    res = run_bass_kernel_spmd(nc, in_maps, list(range(8)), trace=_trace,
                               tmpdir=_tmpdir)
    if _trace:
        _CACHED["last_result"] = res

    out = np.empty((B, S, D), dtype=np.float32)
    for b in range(B):
        acc = res.results[2 * b]["outT"] + res.results[2 * b + 1]["outT"]
        out[b] = acc.T
    return out


# revision 23
# speedup vs baseline: 1.0511x; 1.0511x over previous
"""Trainium2 Bass kernel for nn_MultiHeadAttn (B=4, S=2048, D=1024, H=16).

Sharding: 8 cores = 4 batches x 2 head-groups (tensor-parallel over heads).
Each core computes one batch's attention for 8 of 16 heads (512 of 1024
feature dims) and a partial output projection; the host sums the two
head-group partials per batch (the "all-reduce" of row-parallel Wo).

Device dataflow (matmuls in fp16 with fp32 PSUM accumulation; fp16 keeps
10 mantissa bits so end-to-end error stays ~7e-4 while enabling the fast
weight-load path the PE lacks for fp32/fp32r):
  - Host pre-transposes activations (q/k/v -> [D, S]) and weight slices and
    converts to fp16, so the kernel needs no on-device transposes or
    casting DMAs.
  - QT/KT computed feature-major [512, 2048]; V computed token-major with a
    ones column per head ([128, 8*65] tiles) so the attn@V matmul (M=65)
    also produces the softmax row-sums.
  - Scores computed transposed S^T[k,q] with 2-head row-tiled matmuls
    (K=64 pairs packed at tile_position (0,0)/(64,0)).
  - softmax without max-subtraction (scores/8 ~ N(0,1), exp is safe);
    exp on ScalarE with scale=1/8 fused.
  - Each attention tile emits its 16 exp+next-scores pairs first (the
    ScalarE chain), then filler projection work (V / later-tile Q), then
    the 32 attn@V matmuls consuming a deep P pool — so ScalarE, the
    critical engine, is never queued behind PE filler work.
  - Division: K=1 ones-matmul broadcast of raw row-sums + fast DVE
    reciprocal (reciprocal_approx_fast, ~18-bit) + DVE multiply, inline at
    tile end reading Y straight from PSUM.
  - Output projection consumes X^T directly; bv/bo folded into a single
    host-precomputed effective bias; emitted per token-tile right after
    the last head-pair finishes that tile.
"""
import numpy as np

B, S, D = 4, 2048, 1024
H = 16
DK = 64
G = 2              # head groups (tensor-parallel factor)
DL = D // G        # 512 local feature dims per core
NHL = H // G       # 8 local heads
NJ = NHL // 2      # 4 head pairs
NT = S // 512      # 4 token tiles of 512
NKC = S // 128     # 16 k-token chunks of 128
NDC = D // 128     # 8 d_in chunks
NM = DL // 128     # 4 local out chunks
NMO = D // 128     # 8 output d chunks

_CACHED = {}


def _build_nc():
    import concourse.bass as bass
    import concourse.tile as tile
    from concourse import bacc, mybir

    FP32 = mybir.dt.float32
    FP16 = mybir.dt.float16
    AF = mybir.ActivationFunctionType
    ts = bass.ts

    nc = bacc.Bacc(None, target_bir_lowering=False, debug=False)

    qT_d = nc.dram_tensor("qT", [D, S], FP16, kind="ExternalInput")
    kT_d = nc.dram_tensor("kT", [D, S], FP16, kind="ExternalInput")
    vT_d = nc.dram_tensor("vT", [D, S], FP16, kind="ExternalInput")
    wqT_d = nc.dram_tensor("wqT", [D, DL], FP16, kind="ExternalInput")
    wkT_d = nc.dram_tensor("wkT", [D, DL], FP16, kind="ExternalInput")
    wvT_d = nc.dram_tensor("wvT", [D, DL], FP16, kind="ExternalInput")
    woT_d = nc.dram_tensor("woT", [DL, D], FP16, kind="ExternalInput")
    bq_d = nc.dram_tensor("bq", [NM, 128, 1], FP32, kind="ExternalInput")
    bk_d = nc.dram_tensor("bk", [NM, 128, 1], FP32, kind="ExternalInput")
    bo_d = nc.dram_tensor("bo", [NMO, 128, 1], FP32, kind="ExternalInput")
    out_d = nc.dram_tensor("outT", [D, S], FP32, kind="ExternalOutput")

    with tile.TileContext(nc) as tc:
        with (
            tc.tile_pool(name="const", bufs=1) as const,
            tc.tile_pool(name="wflat", bufs=24) as wflat,
            tc.tile_pool(name="wop", bufs=4) as wop,
            tc.tile_pool(name="qkwin", bufs=16) as qkwin,
            tc.tile_pool(name="vtwin", bufs=12) as vtwin,
            tc.tile_pool(name="big", bufs=1) as big,
            tc.tile_pool(name="vaug", bufs=1) as vaug,
            tc.tile_pool(name="ppool", bufs=18) as ppool,
            tc.tile_pool(name="small", bufs=3) as small,
            tc.tile_pool(name="outst", bufs=3) as outst,
            tc.tile_pool(name="ps_mm", bufs=2, space="PSUM") as ps_mm,
            tc.tile_pool(name="ps_s", bufs=2, space="PSUM") as ps_s,
            tc.tile_pool(name="ps_y", bufs=2, space="PSUM") as ps_y,
        ):
            # ---- constants
            onescols = const.tile([128, NHL, 1], FP16, name="onescols")
            nc.vector.memset(onescols[:], 1.0)
            bq_sb, bk_sb, bo_sb = [], [], []
            for m in range(NM):
                t_ = const.tile([128, 1], FP32, name=f"bq{m}")
                nc.gpsimd.dma_start(t_[:], bq_d[m])
                bq_sb.append(t_)
                t_ = const.tile([128, 1], FP32, name=f"bk{m}")
                nc.gpsimd.dma_start(t_[:], bk_d[m])
                bk_sb.append(t_)
            for m in range(NMO):
                t_ = const.tile([128, 1], FP32, name=f"bo{m}")
                nc.gpsimd.dma_start(t_[:], bo_d[m])
                bo_sb.append(t_)

            # ---- weight tiles (loads emitted in the emission section so
            # the first projection's windows aren't queued behind them)
            wq_sb, wk_sb, wv_sb, wo_sb = [], [], [], []
            for kc in range(NDC):
                wk_sb.append(wflat.tile([128, DL], FP16, tag="w",
                                        name=f"wk{kc}"))
                wq_sb.append(wflat.tile([128, DL], FP16, tag="w",
                                        name=f"wq{kc}"))
                wv_sb.append(wflat.tile([128, DL], FP16, tag="w",
                                        name=f"wv{kc}"))
            for jc in range(NJ):
                wo_sb.append(wop.tile([128, D], FP16, tag="wo",
                                      name=f"wo{jc}"))

            # ---- resident activation tiles (fp16)
            QT = [big.tile([128, S], FP16, name=f"QT{m}") for m in range(NM)]
            KT = [big.tile([128, S], FP16, name=f"KT{m}") for m in range(NM)]
            X = [big.tile([128, S], FP16, name=f"X{j}") for j in range(NJ)]
            VA = [vaug.tile([128, NHL * 65], FP16, name=f"va{c}")
                  for c in range(NKC)]
            va_view = [va[:].rearrange("p (h c) -> p h c", c=65) for va in VA]

            # ---- task emitters -------------------------------------------
            def qk_task(src_d, w_sb, b_sb, dst, t, ms):
                """Project token-tile t of q or k for m-chunks in ms."""
                win = []
                for kc in range(NDC):
                    w_ = qkwin.tile([128, 512], FP16, tag="win",
                                    name=f"win{kc}")
                    nc.sync.dma_start(w_[:], src_d[ts(kc, 128), ts(t, 512)])
                    win.append(w_)
                for m in ms:
                    ps = ps_mm.tile([128, 512], FP32, tag="mm", name="psA")
                    for kc in range(NDC):
                        nc.tensor.matmul(
                            ps[:], w_sb[kc][:, ts(m, 128)], win[kc][:],
                            start=(kc == 0), stop=(kc == NDC - 1))
                    nc.vector.tensor_scalar_add(
                        dst[m][:, ts(t, 512)], ps[:], b_sb[m][:])

            def v_task(c):
                """Project token-chunk c of v into the ones-augmented VA."""
                ps = ps_mm.tile([128, 512], FP32, tag="mm", name="psV")
                for kc in range(NDC):
                    vt = vtwin.tile([128, 128], FP16, tag="vt", name="vt")
                    nc.gpsimd.dma_start(vt[:], vT_d[ts(kc, 128), ts(c, 128)])
                    nc.tensor.matmul(ps[:], vt[:], wv_sb[kc][:],
                                     start=(kc == 0), stop=(kc == NDC - 1))
                ps_v = ps[:].rearrange("p (h c) -> p h c", c=64)
                nc.vector.tensor_copy(va_view[c][:, :, 0:64], ps_v)
                nc.vector.tensor_copy(va_view[c][:, :, 64:65], onescols[:])

            def out_task(t):
                """Output projection for token-tile t (needs all X_j)."""
                for m in range(NMO):
                    ps = ps_mm.tile([128, 512], FP32, tag="mm", name="psO")
                    for j in range(NJ):
                        nc.tensor.matmul(
                            ps[:], wo_sb[j][:, ts(m, 128)],
                            X[j][:, ts(t, 512)],
                            start=(j == 0), stop=(j == NJ - 1))
                    st = outst.tile([128, 512], FP32, tag="st", name="st")
                    nc.vector.tensor_scalar_add(st[:], ps[:], bo_sb[m][:])
                    nc.sync.dma_start(out_d[ts(m, 128), ts(t, 512)], st[:])

            # ---- deferred K/Q projection fillers, (m, t)-granular.
            # Pair j needs KT_j / QT_j (m-chunk j) for all t; m=0 runs
            # upfront, m-chunk j is drained as filler during pair j-1.
            done = set()

            def ensure_proj(which, m, t):
                if (which, m, t) in done:
                    return
                done.add((which, m, t))
                if which == "k":
                    qk_task(kT_d, wk_sb, bk_sb, KT, t, [m])
                else:
                    qk_task(qT_d, wq_sb, bq_sb, QT, t, [m])

            filler_q = [(w, m, t) for m in range(1, NM)
                        for t in range(NT) for w in ("k", "q")]

            def pop_filler():
                while filler_q:
                    w, m, t = filler_q.pop(0)
                    if (w, m, t) not in done:
                        ensure_proj(w, m, t)
                        return

            def attn_tile(j, t):
                """Attention for head-pair j, token-tile t.
                Emission order: the full exp/scores chain, then fillers,
                then the attn@V matmuls, then the normalization."""
                first = (j == 0 and t == 0)
                ys = [ps_y.tile([65, 512], FP32, tag="y", name=f"y{h}")
                      for h in range(2)]

                def scores(k):
                    s_ps = ps_s.tile([128, 1024], FP32, tag="s", name="s")
                    nc.tensor.matmul(
                        s_ps[:, 0:512], KT[j][0:64, ts(k, 128)],
                        QT[j][0:64, ts(t, 512)],
                        start=True, stop=True, tile_position=(0, 0))
                    nc.tensor.matmul(
                        s_ps[:, 512:1024], KT[j][64:128, ts(k, 128)],
                        QT[j][64:128, ts(t, 512)],
                        start=True, stop=True, tile_position=(64, 0))
                    return s_ps

                def a_v(k, p):
                    for h in range(2):
                        nc.tensor.matmul(
                            ys[h][:],
                            VA[k][:, 65 * (2 * j + h):65 * (2 * j + h) + 65],
                            p[:, 512 * h:512 * (h + 1)],
                            start=(k == 0), stop=(k == NKC - 1))

                # attn@V runs two iterations behind exp so it never waits on
                # the exp semaphore round-trip
                if first:
                    v_task(0)
                s_cur = scores(0)
                plag = []
                for k in range(NKC):
                    p = ppool.tile([128, 1024], FP16, tag="p", name="p")
                    nc.scalar.activation(p[:], s_cur[:], AF.Exp, scale=0.125)
                    plag.append((k, p))
                    if k + 1 < NKC:
                        s_cur = scores(k + 1)
                    if first and k + 1 < NKC:
                        v_task(k + 1)
                    elif k in (3, 8, 13):
                        pop_filler()
                    if len(plag) > 2:
                        a_v(*plag.pop(0))
                while plag:
                    a_v(*plag.pop(0))

                # normalization straight from PSUM; the row-sum broadcast
                # runs on the otherwise-idle GpSimd engine so the PE rolls
                # directly into the next tile
                for h in range(2):
                    rs = small.tile([1, 512], FP32, tag="rs", name="rs")
                    nc.vector.tensor_copy(rs[:], ys[h][64:65, :])
                    rbb = small.tile([64, 512], FP32, tag="rbb", name="rbb")
                    nc.gpsimd.partition_broadcast(rbb[:], rs[:], channels=64)
                    ri = small.tile([64, 512], FP32, tag="ri", name="ri")
                    nc.vector.reciprocal_approx_fast(ri[:], rbb[:])
                    nc.vector.tensor_mul(
                        X[j][64 * h:64 * h + 64, ts(t, 512)],
                        ys[h][0:64, :], ri[:])

            # ---- emission ------------------------------------------------
            for kc in range(NDC):
                nc.sync.dma_start(wk_sb[kc][:], wkT_d[ts(kc, 128), :])
            for t in range(NT):
                ensure_proj("k", 0, t)
            for kc in range(NDC):
                nc.sync.dma_start(wq_sb[kc][:], wqT_d[ts(kc, 128), :])
            for t in range(NT):
                ensure_proj("q", 0, t)
            for kc in range(NDC):
                nc.sync.dma_start(wv_sb[kc][:], wvT_d[ts(kc, 128), :])
            for jc in range(NJ):
                nc.sync.dma_start(wo_sb[jc][:], woT_d[ts(jc, 128), :])

            for j in range(NJ):
                for t in range(NT):
                    ensure_proj("k", j, t)
                    ensure_proj("q", j, t)
                    attn_tile(j, t)
                    if j == NJ - 1:
                        out_task(t)

    nc.compile()
    return nc


def _prep_in_maps(q, k, v, Wq, bq, Wk, bk, Wv, bv, Wo, bo):
    f16 = np.float16
    in_maps = []
    for core in range(8):
        b, g = divmod(core, G)
        rows = slice(DL * g, DL * (g + 1))
        bo_eff = Wo[:, rows].astype(np.float32) @ bv[rows].astype(np.float32)
        if g == 0:
            bo_eff = bo_eff + bo
        in_maps.append({
            "qT": np.ascontiguousarray(q[b].T.astype(f16)),
            "kT": np.ascontiguousarray(k[b].T.astype(f16)),
            "vT": np.ascontiguousarray(v[b].T.astype(f16)),
            "wqT": np.ascontiguousarray(Wq[rows, :].T.astype(f16)),
            "wkT": np.ascontiguousarray(Wk[rows, :].T.astype(f16)),
            "wvT": np.ascontiguousarray(Wv[rows, :].T.astype(f16)),
            "woT": np.ascontiguousarray(Wo[:, rows].T.astype(f16)),
            "bq": np.ascontiguousarray(bq[rows].reshape(NM, 128, 1)),
            "bk": np.ascontiguousarray(bk[rows].reshape(NM, 128, 1)),
            "bo": np.ascontiguousarray(
                bo_eff.astype(np.float32).reshape(NMO, 128, 1)),
        })
    return in_maps


def kernel(q, k, v, mask, Wq, bq, Wk, bk, Wv, bv, Wo, bo,
           _trace=False, _tmpdir=None):
    from concourse.bass_utils import run_bass_kernel_spmd

    q, k, v = (np.asarray(x, dtype=np.float32) for x in (q, k, v))
    Wq, bq, Wk, bk, Wv, bv, Wo, bo = (
        np.asarray(x, dtype=np.float32)
        for x in (Wq, bq, Wk, bk, Wv, bv, Wo, bo))

    if "nc" not in _CACHED:
        _CACHED["nc"] = _build_nc()
    nc = _CACHED["nc"]

    in_maps = _prep_in_maps(q, k, v, Wq, bq, Wk, bk, Wv, bv, Wo, bo)
    res = run_bass_kernel_spmd(nc, in_maps, list(range(8)), trace=_trace,
                               tmpdir=_tmpdir)
    if _trace:
        _CACHED["last_result"] = res

    out = np.empty((B, S, D), dtype=np.float32)
    for b in range(B):
        acc = res.results[2 * b]["outT"] + res.results[2 * b + 1]["outT"]
        out[b] = acc.T
    return out


# revision 24
# speedup vs baseline: 1.2267x; 1.1671x over previous
"""Trainium2 Bass kernel for nn_MultiHeadAttn (B=4, S=2048, D=1024, H=16).

Sharding: 8 cores = 4 batches x 2 head-groups (tensor-parallel over heads).
Each core computes one batch's attention for 8 of 16 heads (512 of 1024
feature dims) and a partial output projection; the host sums the two
head-group partials per batch (the "all-reduce" of row-parallel Wo).

Device dataflow (matmuls in fp16 with fp32 PSUM accumulation; fp16 keeps
10 mantissa bits so end-to-end error stays ~7e-4 while enabling the fast
weight-load path the PE lacks for fp32/fp32r):
  - Host pre-transposes activations (q/k/v -> [D, S]) and weight slices and
    converts to fp16, so the kernel needs no on-device transposes or
    casting DMAs.
  - QT/KT computed feature-major [512, 2048]; V computed token-major with a
    ones column per head ([128, 8*65] tiles) so the attn@V matmul (M=65)
    also produces the softmax row-sums.
  - Scores computed transposed S^T[k,q] with 2-head row-tiled matmuls
    (K=64 pairs packed at tile_position (0,0)/(64,0)).
  - softmax without max-subtraction (scores/8 ~ N(0,1), exp is safe);
    exp on ScalarE with scale=1/8 fused.
  - Each attention tile emits its 16 exp+next-scores pairs first (the
    ScalarE chain), then filler projection work (V / later-tile Q), then
    the 32 attn@V matmuls consuming a deep P pool — so ScalarE, the
    critical engine, is never queued behind PE filler work.
  - Division: K=1 ones-matmul broadcast of raw row-sums + fast DVE
    reciprocal (reciprocal_approx_fast, ~18-bit) + DVE multiply, inline at
    tile end reading Y straight from PSUM.
  - Output projection consumes X^T directly; bv/bo folded into a single
    host-precomputed effective bias; emitted per token-tile right after
    the last head-pair finishes that tile.
"""
import numpy as np

B, S, D = 4, 2048, 1024
H = 16
DK = 64
G = 2              # head groups (tensor-parallel factor)
DL = D // G        # 512 local feature dims per core
NHL = H // G       # 8 local heads
NJ = NHL // 2      # 4 head pairs
NT = S // 512      # 4 token tiles of 512
NKC = S // 128     # 16 k-token chunks of 128
NDC = D // 128     # 8 d_in chunks
NM = DL // 128     # 4 local out chunks
NMO = D // 128     # 8 output d chunks

_CACHED = {}


def _build_nc():
    import concourse.bass as bass
    import concourse.tile as tile
    from concourse import bacc, mybir

    FP32 = mybir.dt.float32
    FP16 = mybir.dt.float16
    AF = mybir.ActivationFunctionType
    ts = bass.ts

    nc = bacc.Bacc(None, target_bir_lowering=False, debug=False)

    qT_d = nc.dram_tensor("qT", [D, S], FP16, kind="ExternalInput")
    kT_d = nc.dram_tensor("kT", [D, S], FP16, kind="ExternalInput")
    vT_d = nc.dram_tensor("vT", [D, S], FP16, kind="ExternalInput")
    wqT_d = nc.dram_tensor("wqT", [D, DL], FP16, kind="ExternalInput")
    wkT_d = nc.dram_tensor("wkT", [D, DL], FP16, kind="ExternalInput")
    wvT_d = nc.dram_tensor("wvT", [D, DL], FP16, kind="ExternalInput")
    woT_d = nc.dram_tensor("woT", [DL, D], FP16, kind="ExternalInput")
    bq_d = nc.dram_tensor("bq", [NM, 128, 1], FP32, kind="ExternalInput")
    bk_d = nc.dram_tensor("bk", [NM, 128, 1], FP32, kind="ExternalInput")
    bo_d = nc.dram_tensor("bo", [NMO, 128, 1], FP32, kind="ExternalInput")
    out_d = nc.dram_tensor("outT", [D, S], FP32, kind="ExternalOutput")

    with tile.TileContext(nc) as tc:
        with (
            tc.tile_pool(name="const", bufs=1) as const,
            tc.tile_pool(name="wflat", bufs=24) as wflat,
            tc.tile_pool(name="wop", bufs=4) as wop,
            tc.tile_pool(name="qkwin", bufs=16) as qkwin,
            tc.tile_pool(name="vtwin", bufs=12) as vtwin,
            tc.tile_pool(name="big", bufs=1) as big,
            tc.tile_pool(name="vaug", bufs=1) as vaug,
            tc.tile_pool(name="ppool", bufs=18) as ppool,
            tc.tile_pool(name="small", bufs=3) as small,
            tc.tile_pool(name="outst", bufs=3) as outst,
            tc.tile_pool(name="ps_mm", bufs=2, space="PSUM") as ps_mm,
            tc.tile_pool(name="ps_s", bufs=2, space="PSUM") as ps_s,
            tc.tile_pool(name="ps_y", bufs=2, space="PSUM") as ps_y,
        ):
            # ---- constants
            onescols = const.tile([128, NHL, 1], FP16, name="onescols")
            nc.vector.memset(onescols[:], 1.0)
            bq_sb, bk_sb, bo_sb = [], [], []
            for m in range(NM):
                t_ = const.tile([128, 1], FP32, name=f"bq{m}")
                nc.gpsimd.dma_start(t_[:], bq_d[m])
                bq_sb.append(t_)
                t_ = const.tile([128, 1], FP32, name=f"bk{m}")
                nc.gpsimd.dma_start(t_[:], bk_d[m])
                bk_sb.append(t_)
            for m in range(NMO):
                t_ = const.tile([128, 1], FP32, name=f"bo{m}")
                nc.gpsimd.dma_start(t_[:], bo_d[m])
                bo_sb.append(t_)

            # ---- weight tiles (loads emitted in the emission section so
            # the first projection's windows aren't queued behind them)
            wq_sb, wk_sb, wv_sb, wo_sb = [], [], [], []
            for kc in range(NDC):
                wk_sb.append(wflat.tile([128, DL], FP16, tag="w",
                                        name=f"wk{kc}"))
                wq_sb.append(wflat.tile([128, DL], FP16, tag="w",
                                        name=f"wq{kc}"))
                wv_sb.append(wflat.tile([128, DL], FP16, tag="w",
                                        name=f"wv{kc}"))
            for jc in range(NJ):
                wo_sb.append(wop.tile([128, D], FP16, tag="wo",
                                      name=f"wo{jc}"))

            # ---- resident activation tiles (fp16)
            QT = [big.tile([128, S], FP16, name=f"QT{m}") for m in range(NM)]
            KT = [big.tile([128, S], FP16, name=f"KT{m}") for m in range(NM)]
            X = [big.tile([128, S], FP16, name=f"X{j}") for j in range(NJ)]
            VA = [vaug.tile([128, NHL * 65], FP16, name=f"va{c}")
                  for c in range(NKC)]
            va_view = [va[:].rearrange("p (h c) -> p h c", c=65) for va in VA]

            # ---- task emitters -------------------------------------------
            def qk_task(src_d, w_sb, b_sb, dst, t, ms):
                """Project token-tile t of q or k for m-chunks in ms."""
                win = []
                for kc in range(NDC):
                    w_ = qkwin.tile([128, 512], FP16, tag="win",
                                    name=f"win{kc}")
                    nc.sync.dma_start(w_[:], src_d[ts(kc, 128), ts(t, 512)])
                    win.append(w_)
                for m in ms:
                    ps = ps_mm.tile([128, 512], FP32, tag="mm", name="psA")
                    for kc in range(NDC):
                        nc.tensor.matmul(
                            ps[:], w_sb[kc][:, ts(m, 128)], win[kc][:],
                            start=(kc == 0), stop=(kc == NDC - 1))
                    nc.vector.tensor_scalar_add(
                        dst[m][:, ts(t, 512)], ps[:], b_sb[m][:])

            def v_task(c):
                """Project token-chunk c of v into the ones-augmented VA."""
                ps = ps_mm.tile([128, 512], FP32, tag="mm", name="psV")
                for kc in range(NDC):
                    vt = vtwin.tile([128, 128], FP16, tag="vt", name="vt")
                    nc.sync.dma_start(vt[:], vT_d[ts(kc, 128), ts(c, 128)])
                    nc.tensor.matmul(ps[:], vt[:], wv_sb[kc][:],
                                     start=(kc == 0), stop=(kc == NDC - 1))
                ps_v = ps[:].rearrange("p (h c) -> p h c", c=64)
                nc.vector.tensor_copy(va_view[c][:, :, 0:64], ps_v)
                nc.vector.tensor_copy(va_view[c][:, :, 64:65], onescols[:])

            def out_task(t):
                """Output projection for token-tile t (needs all X_j)."""
                for m in range(NMO):
                    ps = ps_mm.tile([128, 512], FP32, tag="mm", name="psO")
                    for j in range(NJ):
                        nc.tensor.matmul(
                            ps[:], wo_sb[j][:, ts(m, 128)],
                            X[j][:, ts(t, 512)],
                            start=(j == 0), stop=(j == NJ - 1))
                    st = outst.tile([128, 512], FP32, tag="st", name="st")
                    nc.vector.tensor_scalar_add(st[:], ps[:], bo_sb[m][:])
                    nc.sync.dma_start(out_d[ts(m, 128), ts(t, 512)], st[:])

            # ---- deferred K/Q projection fillers, (m, t)-granular.
            # Pair j needs KT_j / QT_j (m-chunk j) for all t; m=0 runs
            # upfront, m-chunk j is drained as filler during pair j-1.
            done = set()

            def ensure_proj(which, m, t):
                if (which, m, t) in done:
                    return
                done.add((which, m, t))
                if which == "k":
                    qk_task(kT_d, wk_sb, bk_sb, KT, t, [m])
                else:
                    qk_task(qT_d, wq_sb, bq_sb, QT, t, [m])

            filler_q = [(w, m, t) for m in range(1, NM)
                        for t in range(NT) for w in ("k", "q")]

            def pop_filler():
                while filler_q:
                    w, m, t = filler_q.pop(0)
                    if (w, m, t) not in done:
                        ensure_proj(w, m, t)
                        return

            def attn_tile(j, t):
                """Attention for head-pair j, token-tile t.
                Emission order: the full exp/scores chain, then fillers,
                then the attn@V matmuls, then the normalization."""
                first = (j == 0 and t == 0)
                ys = [ps_y.tile([65, 512], FP32, tag="y", name=f"y{h}")
                      for h in range(2)]

                def scores(k):
                    s_ps = ps_s.tile([128, 1024], FP32, tag="s", name="s")
                    nc.tensor.matmul(
                        s_ps[:, 0:512], KT[j][0:64, ts(k, 128)],
                        QT[j][0:64, ts(t, 512)],
                        start=True, stop=True, tile_position=(0, 0))
                    nc.tensor.matmul(
                        s_ps[:, 512:1024], KT[j][64:128, ts(k, 128)],
                        QT[j][64:128, ts(t, 512)],
                        start=True, stop=True, tile_position=(64, 0))
                    return s_ps

                def a_v(k, p):
                    for h in range(2):
                        nc.tensor.matmul(
                            ys[h][:],
                            VA[k][:, 65 * (2 * j + h):65 * (2 * j + h) + 65],
                            p[:, 512 * h:512 * (h + 1)],
                            start=(k == 0), stop=(k == NKC - 1))

                # attn@V runs two iterations behind exp so it never waits on
                # the exp semaphore round-trip
                if first:
                    v_task(0)
                s_cur = scores(0)
                plag = []
                for k in range(NKC):
                    p = ppool.tile([128, 1024], FP16, tag="p", name="p")
                    nc.scalar.activation(p[:], s_cur[:], AF.Exp, scale=0.125)
                    plag.append((k, p))
                    if k + 1 < NKC:
                        s_cur = scores(k + 1)
                    if first and k + 1 < NKC:
                        v_task(k + 1)
                    elif k in (3, 8, 13):
                        pop_filler()
                    if len(plag) > 2:
                        a_v(*plag.pop(0))
                while plag:
                    a_v(*plag.pop(0))

                # normalization straight from PSUM; the row-sum broadcast
                # runs on the otherwise-idle GpSimd engine so the PE rolls
                # directly into the next tile
                for h in range(2):
                    rs = small.tile([1, 512], FP32, tag="rs", name="rs")
                    nc.vector.tensor_copy(rs[:], ys[h][64:65, :])
                    rbb = small.tile([64, 512], FP32, tag="rbb", name="rbb")
                    nc.gpsimd.partition_broadcast(rbb[:], rs[:], channels=64)
                    ri = small.tile([64, 512], FP32, tag="ri", name="ri")
                    nc.vector.reciprocal_approx_fast(ri[:], rbb[:])
                    nc.vector.tensor_mul(
                        X[j][64 * h:64 * h + 64, ts(t, 512)],
                        ys[h][0:64, :], ri[:])

            # ---- emission ------------------------------------------------
            for kc in range(NDC):
                nc.sync.dma_start(wk_sb[kc][:], wkT_d[ts(kc, 128), :])
            for t in range(NT):
                ensure_proj("k", 0, t)
            for kc in range(NDC):
                nc.sync.dma_start(wq_sb[kc][:], wqT_d[ts(kc, 128), :])
            for t in range(NT):
                ensure_proj("q", 0, t)
            for kc in range(NDC):
                nc.sync.dma_start(wv_sb[kc][:], wvT_d[ts(kc, 128), :])
            for jc in range(NJ):
                nc.sync.dma_start(wo_sb[jc][:], woT_d[ts(jc, 128), :])

            for j in range(NJ):
                for t in range(NT):
                    ensure_proj("k", j, t)
                    ensure_proj("q", j, t)
                    attn_tile(j, t)
                    if j == NJ - 1:
                        out_task(t)

    nc.compile()
    return nc


def _prep_in_maps(q, k, v, Wq, bq, Wk, bk, Wv, bv, Wo, bo):
    f16 = np.float16
    in_maps = []
    for core in range(8):
        b, g = divmod(core, G)
        rows = slice(DL * g, DL * (g + 1))
        bo_eff = Wo[:, rows].astype(np.float32) @ bv[rows].astype(np.float32)
        if g == 0:
            bo_eff = bo_eff + bo
        in_maps.append({
            "qT": np.ascontiguousarray(q[b].T.astype(f16)),
            "kT": np.ascontiguousarray(k[b].T.astype(f16)),
            "vT": np.ascontiguousarray(v[b].T.astype(f16)),
            "wqT": np.ascontiguousarray(Wq[rows, :].T.astype(f16)),
            "wkT": np.ascontiguousarray(Wk[rows, :].T.astype(f16)),
            "wvT": np.ascontiguousarray(Wv[rows, :].T.astype(f16)),
            "woT": np.ascontiguousarray(Wo[:, rows].T.astype(f16)),
            "bq": np.ascontiguousarray(bq[rows].reshape(NM, 128, 1)),
            "bk": np.ascontiguousarray(bk[rows].reshape(NM, 128, 1)),
            "bo": np.ascontiguousarray(
                bo_eff.astype(np.float32).reshape(NMO, 128, 1)),
        })
    return in_maps


def kernel(q, k, v, mask, Wq, bq, Wk, bk, Wv, bv, Wo, bo,
           _trace=False, _tmpdir=None):
    from concourse.bass_utils import run_bass_kernel_spmd

    q, k, v = (np.asarray(x, dtype=np.float32) for x in (q, k, v))
    Wq, bq, Wk, bk, Wv, bv, Wo, bo = (
        np.asarray(x, dtype=np.float32)
        for x in (Wq, bq, Wk, bk, Wv, bv, Wo, bo))

    if "nc" not in _CACHED:
        _CACHED["nc"] = _build_nc()
    nc = _CACHED["nc"]

    in_maps = _prep_in_maps(q, k, v, Wq, bq, Wk, bk, Wv, bv, Wo, bo)
    res = run_bass_kernel_spmd(nc, in_maps, list(range(8)), trace=_trace,
                               tmpdir=_tmpdir)
    if _trace:
        _CACHED["last_result"] = res

    out = np.empty((B, S, D), dtype=np.float32)
    for b in range(B):
        acc = res.results[2 * b]["outT"] + res.results[2 * b + 1]["outT"]
        out[b] = acc.T
    return out


# revision 25
# speedup vs baseline: 1.2670x; 1.0328x over previous
"""Trainium2 Bass kernel for nn_MultiHeadAttn (B=4, S=2048, D=1024, H=16).

Sharding: 8 cores = 4 batches x 2 head-groups (tensor-parallel over heads).
Each core computes one batch's attention for 8 of 16 heads (512 of 1024
feature dims) and a partial output projection; the host sums the two
head-group partials per batch (the "all-reduce" of row-parallel Wo).

Device dataflow (matmuls in fp16 with fp32 PSUM accumulation; fp16 keeps
10 mantissa bits so end-to-end error stays ~7e-4 while enabling the fast
weight-load path the PE lacks for fp32/fp32r):
  - Host pre-transposes activations (q/k/v -> [D, S]) and weight slices and
    converts to fp16, so the kernel needs no on-device transposes or
    casting DMAs.
  - QT/KT computed feature-major [512, 2048]; V computed token-major with a
    ones column per head ([128, 8*65] tiles) so the attn@V matmul (M=65)
    also produces the softmax row-sums.
  - Scores computed transposed S^T[k,q] with 2-head row-tiled matmuls
    (K=64 pairs packed at tile_position (0,0)/(64,0)).
  - softmax without max-subtraction (scores/8 ~ N(0,1), exp is safe);
    exp on ScalarE with scale=1/8 fused.
  - Each attention tile emits its 16 exp+next-scores pairs first (the
    ScalarE chain), then filler projection work (V / later-tile Q), then
    the 32 attn@V matmuls consuming a deep P pool — so ScalarE, the
    critical engine, is never queued behind PE filler work.
  - Division: K=1 ones-matmul broadcast of raw row-sums + fast DVE
    reciprocal (reciprocal_approx_fast, ~18-bit) + DVE multiply, inline at
    tile end reading Y straight from PSUM.
  - Output projection consumes X^T directly; bv/bo folded into a single
    host-precomputed effective bias; emitted per token-tile right after
    the last head-pair finishes that tile.
"""
import numpy as np

B, S, D = 4, 2048, 1024
H = 16
DK = 64
G = 2              # head groups (tensor-parallel factor)
DL = D // G        # 512 local feature dims per core
NHL = H // G       # 8 local heads
NJ = NHL // 2      # 4 head pairs
NT = S // 512      # 4 token tiles of 512
NKC = S // 128     # 16 k-token chunks of 128
NDC = D // 128     # 8 d_in chunks
NM = DL // 128     # 4 local out chunks
NMO = D // 128     # 8 output d chunks

_CACHED = {}


def _build_nc():
    import concourse.bass as bass
    import concourse.tile as tile
    from concourse import bacc, mybir

    FP32 = mybir.dt.float32
    FP16 = mybir.dt.float16
    AF = mybir.ActivationFunctionType
    ts = bass.ts

    nc = bacc.Bacc(None, target_bir_lowering=False, debug=False)

    qT_d = nc.dram_tensor("qT", [D, S], FP16, kind="ExternalInput")
    kT_d = nc.dram_tensor("kT", [D, S], FP16, kind="ExternalInput")
    vT_d = nc.dram_tensor("vT", [D, S], FP16, kind="ExternalInput")
    wqT_d = nc.dram_tensor("wqT", [D, DL], FP16, kind="ExternalInput")
    wkT_d = nc.dram_tensor("wkT", [D, DL], FP16, kind="ExternalInput")
    wvT_d = nc.dram_tensor("wvT", [D, DL], FP16, kind="ExternalInput")
    woT_d = nc.dram_tensor("woT", [DL, D], FP16, kind="ExternalInput")
    bq_d = nc.dram_tensor("bq", [NM, 128, 1], FP32, kind="ExternalInput")
    bk_d = nc.dram_tensor("bk", [NM, 128, 1], FP32, kind="ExternalInput")
    bo_d = nc.dram_tensor("bo", [NMO, 128, 1], FP32, kind="ExternalInput")
    out_d = nc.dram_tensor("outT", [D, S], FP32, kind="ExternalOutput")

    with tile.TileContext(nc) as tc:
        with (
            tc.tile_pool(name="const", bufs=1) as const,
            tc.tile_pool(name="wflat", bufs=24) as wflat,
            tc.tile_pool(name="wop", bufs=4) as wop,
            tc.tile_pool(name="qkwin", bufs=24) as qkwin,
            tc.tile_pool(name="vtwin", bufs=16) as vtwin,
            tc.tile_pool(name="big", bufs=1) as big,
            tc.tile_pool(name="vaug", bufs=1) as vaug,
            tc.tile_pool(name="ppool", bufs=18) as ppool,
            tc.tile_pool(name="small", bufs=3) as small,
            tc.tile_pool(name="outst", bufs=3) as outst,
            tc.tile_pool(name="ps_mm", bufs=2, space="PSUM") as ps_mm,
            tc.tile_pool(name="ps_s", bufs=2, space="PSUM") as ps_s,
            tc.tile_pool(name="ps_y", bufs=2, space="PSUM") as ps_y,
        ):
            # ---- constants
            onescols = const.tile([128, NHL, 1], FP16, name="onescols")
            nc.vector.memset(onescols[:], 1.0)
            bq_sb, bk_sb, bo_sb = [], [], []
            for m in range(NM):
                t_ = const.tile([128, 1], FP32, name=f"bq{m}")
                nc.gpsimd.dma_start(t_[:], bq_d[m])
                bq_sb.append(t_)
                t_ = const.tile([128, 1], FP32, name=f"bk{m}")
                nc.gpsimd.dma_start(t_[:], bk_d[m])
                bk_sb.append(t_)
            for m in range(NMO):
                t_ = const.tile([128, 1], FP32, name=f"bo{m}")
                nc.gpsimd.dma_start(t_[:], bo_d[m])
                bo_sb.append(t_)

            # ---- weight tiles (loads emitted in the emission section so
            # the first projection's windows aren't queued behind them)
            wq_sb, wk_sb, wv_sb, wo_sb = [], [], [], []
            for kc in range(NDC):
                wk_sb.append(wflat.tile([128, DL], FP16, tag="w",
                                        name=f"wk{kc}"))
                wq_sb.append(wflat.tile([128, DL], FP16, tag="w",
                                        name=f"wq{kc}"))
                wv_sb.append(wflat.tile([128, DL], FP16, tag="w",
                                        name=f"wv{kc}"))
            for jc in range(NJ):
                wo_sb.append(wop.tile([128, D], FP16, tag="wo",
                                      name=f"wo{jc}"))

            # ---- resident activation tiles (fp16)
            QT = [big.tile([128, S], FP16, name=f"QT{m}") for m in range(NM)]
            KT = [big.tile([128, S], FP16, name=f"KT{m}") for m in range(NM)]
            X = [big.tile([128, S], FP16, name=f"X{j}") for j in range(NJ)]
            VA = [vaug.tile([128, NHL * 65], FP16, name=f"va{c}")
                  for c in range(NKC)]
            va_view = [va[:].rearrange("p (h c) -> p h c", c=65) for va in VA]

            # ---- task emitters -------------------------------------------
            def qk_task(src_d, w_sb, b_sb, dst, t, ms):
                """Project token-tile t of q or k for m-chunks in ms."""
                win = []
                for kc in range(NDC):
                    w_ = qkwin.tile([128, 512], FP16, tag="win",
                                    name=f"win{kc}")
                    nc.sync.dma_start(w_[:], src_d[ts(kc, 128), ts(t, 512)])
                    win.append(w_)
                for m in ms:
                    ps = ps_mm.tile([128, 512], FP32, tag="mm", name="psA")
                    for kc in range(NDC):
                        nc.tensor.matmul(
                            ps[:], w_sb[kc][:, ts(m, 128)], win[kc][:],
                            start=(kc == 0), stop=(kc == NDC - 1))
                    nc.vector.tensor_scalar_add(
                        dst[m][:, ts(t, 512)], ps[:], b_sb[m][:])

            def v_task(c):
                """Project token-chunk c of v into the ones-augmented VA."""
                ps = ps_mm.tile([128, 512], FP32, tag="mm", name="psV")
                for kc in range(NDC):
                    vt = vtwin.tile([128, 128], FP16, tag="vt", name="vt")
                    nc.sync.dma_start(vt[:], vT_d[ts(kc, 128), ts(c, 128)])
                    nc.tensor.matmul(ps[:], vt[:], wv_sb[kc][:],
                                     start=(kc == 0), stop=(kc == NDC - 1))
                ps_v = ps[:].rearrange("p (h c) -> p h c", c=64)
                nc.vector.tensor_copy(va_view[c][:, :, 0:64], ps_v)
                nc.vector.tensor_copy(va_view[c][:, :, 64:65], onescols[:])

            def out_task(t):
                """Output projection for token-tile t (needs all X_j)."""
                for m in range(NMO):
                    ps = ps_mm.tile([128, 512], FP32, tag="mm", name="psO")
                    for j in range(NJ):
                        nc.tensor.matmul(
                            ps[:], wo_sb[j][:, ts(m, 128)],
                            X[j][:, ts(t, 512)],
                            start=(j == 0), stop=(j == NJ - 1))
                    st = outst.tile([128, 512], FP32, tag="st", name="st")
                    nc.vector.tensor_scalar_add(st[:], ps[:], bo_sb[m][:])
                    nc.sync.dma_start(out_d[ts(m, 128), ts(t, 512)], st[:])

            # ---- deferred K/Q projection fillers, (m, t)-granular.
            # Pair j needs KT_j / QT_j (m-chunk j) for all t; m=0 runs
            # upfront, m-chunk j is drained as filler during pair j-1.
            done = set()

            def ensure_proj(which, m, t):
                if (which, m, t) in done:
                    return
                done.add((which, m, t))
                if which == "k":
                    qk_task(kT_d, wk_sb, bk_sb, KT, t, [m])
                else:
                    qk_task(qT_d, wq_sb, bq_sb, QT, t, [m])

            filler_q = [(w, m, t) for m in range(1, NM)
                        for t in range(NT) for w in ("k", "q")]

            def pop_filler():
                while filler_q:
                    w, m, t = filler_q.pop(0)
                    if (w, m, t) not in done:
                        ensure_proj(w, m, t)
                        return

            def attn_tile(j, t):
                """Attention for head-pair j, token-tile t.
                Emission order: the full exp/scores chain, then fillers,
                then the attn@V matmuls, then the normalization."""
                first = (j == 0 and t == 0)
                ys = [ps_y.tile([65, 512], FP32, tag="y", name=f"y{h}")
                      for h in range(2)]

                def scores(k):
                    s_ps = ps_s.tile([128, 1024], FP32, tag="s", name="s")
                    nc.tensor.matmul(
                        s_ps[:, 0:512], KT[j][0:64, ts(k, 128)],
                        QT[j][0:64, ts(t, 512)],
                        start=True, stop=True, tile_position=(0, 0))
                    nc.tensor.matmul(
                        s_ps[:, 512:1024], KT[j][64:128, ts(k, 128)],
                        QT[j][64:128, ts(t, 512)],
                        start=True, stop=True, tile_position=(64, 0))
                    return s_ps

                def a_v(k, p):
                    for h in range(2):
                        nc.tensor.matmul(
                            ys[h][:],
                            VA[k][:, 65 * (2 * j + h):65 * (2 * j + h) + 65],
                            p[:, 512 * h:512 * (h + 1)],
                            start=(k == 0), stop=(k == NKC - 1))

                # attn@V runs two iterations behind exp so it never waits on
                # the exp semaphore round-trip
                if first:
                    v_task(0)
                s_cur = scores(0)
                plag = []
                for k in range(NKC):
                    p = ppool.tile([128, 1024], FP16, tag="p", name="p")
                    nc.scalar.activation(p[:], s_cur[:], AF.Exp, scale=0.125)
                    plag.append((k, p))
                    if k + 1 < NKC:
                        s_cur = scores(k + 1)
                    if first and k + 1 < NKC:
                        v_task(k + 1)
                    elif k in (3, 8, 13):
                        pop_filler()
                    if len(plag) > 3:
                        a_v(*plag.pop(0))
                while plag:
                    a_v(*plag.pop(0))

                # normalization straight from PSUM; the row-sum broadcast
                # runs on the otherwise-idle GpSimd engine so the PE rolls
                # directly into the next tile
                for h in range(2):
                    rs = small.tile([1, 512], FP32, tag="rs", name="rs")
                    nc.vector.tensor_copy(rs[:], ys[h][64:65, :])
                    rbb = small.tile([64, 512], FP32, tag="rbb", name="rbb")
                    nc.gpsimd.partition_broadcast(rbb[:], rs[:], channels=64)
                    ri = small.tile([64, 512], FP32, tag="ri", name="ri")
                    nc.vector.reciprocal_approx_fast(ri[:], rbb[:])
                    nc.vector.tensor_mul(
                        X[j][64 * h:64 * h + 64, ts(t, 512)],
                        ys[h][0:64, :], ri[:])

            # ---- emission ------------------------------------------------
            for kc in range(NDC):
                nc.gpsimd.dma_start(wk_sb[kc][:], wkT_d[ts(kc, 128), :])
            for t in range(NT):
                ensure_proj("k", 0, t)
            for kc in range(NDC):
                nc.gpsimd.dma_start(wq_sb[kc][:], wqT_d[ts(kc, 128), :])
            for t in range(NT):
                ensure_proj("q", 0, t)
            for kc in range(NDC):
                nc.gpsimd.dma_start(wv_sb[kc][:], wvT_d[ts(kc, 128), :])
            for jc in range(NJ):
                nc.gpsimd.dma_start(wo_sb[jc][:], woT_d[ts(jc, 128), :])

            for j in range(NJ):
                for t in range(NT):
                    ensure_proj("k", j, t)
                    ensure_proj("q", j, t)
                    attn_tile(j, t)
                    if j == NJ - 1:
                        out_task(t)

    nc.compile()
    return nc


def _prep_in_maps(q, k, v, Wq, bq, Wk, bk, Wv, bv, Wo, bo):
    f16 = np.float16
    in_maps = []
    for core in range(8):
        b, g = divmod(core, G)
        rows = slice(DL * g, DL * (g + 1))
        bo_eff = Wo[:, rows].astype(np.float32) @ bv[rows].astype(np.float32)
        if g == 0:
            bo_eff = bo_eff + bo
        in_maps.append({
            "qT": np.ascontiguousarray(q[b].T.astype(f16)),
            "kT": np.ascontiguousarray(k[b].T.astype(f16)),
            "vT": np.ascontiguousarray(v[b].T.astype(f16)),
            "wqT": np.ascontiguousarray(Wq[rows, :].T.astype(f16)),
            "wkT": np.ascontiguousarray(Wk[rows, :].T.astype(f16)),
            "wvT": np.ascontiguousarray(Wv[rows, :].T.astype(f16)),
            "woT": np.ascontiguousarray(Wo[:, rows].T.astype(f16)),
            "bq": np.ascontiguousarray(bq[rows].reshape(NM, 128, 1)),
            "bk": np.ascontiguousarray(bk[rows].reshape(NM, 128, 1)),
            "bo": np.ascontiguousarray(
                bo_eff.astype(np.float32).reshape(NMO, 128, 1)),
        })
    return in_maps


def kernel(q, k, v, mask, Wq, bq, Wk, bk, Wv, bv, Wo, bo,
           _trace=False, _tmpdir=None):
    from concourse.bass_utils import run_bass_kernel_spmd

    q, k, v = (np.asarray(x, dtype=np.float32) for x in (q, k, v))
    Wq, bq, Wk, bk, Wv, bv, Wo, bo = (
        np.asarray(x, dtype=np.float32)
        for x in (Wq, bq, Wk, bk, Wv, bv, Wo, bo))

    if "nc" not in _CACHED:
        _CACHED["nc"] = _build_nc()
    nc = _CACHED["nc"]

    in_maps = _prep_in_maps(q, k, v, Wq, bq, Wk, bk, Wv, bv, Wo, bo)
    res = run_bass_kernel_spmd(nc, in_maps, list(range(8)), trace=_trace,
                               tmpdir=_tmpdir)
    if _trace:
        _CACHED["last_result"] = res

    out = np.empty((B, S, D), dtype=np.float32)
    for b in range(B):
        acc = res.results[2 * b]["outT"] + res.results[2 * b + 1]["outT"]
        out[b] = acc.T
    return out
